# revision 28
# baseline (speedup 1.0000x reference)
"""Chunked bf16 kernel with per-chunk ncfw AllGathers (16/32/16 batches).

Design (from r1-r8 NTFF traces):
- the feature stream runs near the per-NC HBM cap when nothing gates it:
  per-batch DMAs alternate the two HWDGE rings; nothing
  collective-adjacent ever rides those rings mid-stream (ring FIFO would
  stall the stream on a data dependency)
- ncfw mesh AllGather cost grows steeply with payload and has huge
  run-to-run variance (16KB: 5-26us, 32KB: 10-127us), so: a SMALL first
  chunk triggers AG_a by ~120us (even a pathological AG_a finishes
  before AG_b's data exists -- collective_compute blocks the issuing
  gpsimd engine until completion, so a late AG cascades), and the
  exposed final AG carries only 16KB
- DVE reduce throughput is layout-sensitive (ft pool needs 8 bufs for
  1.04 cyc/elem) and DVE trails the stream by ~one delivery quantum, so
  the last chunk streams half-batches and finishes with multi-batch
  j-group units ([4,4,4,2,2]) whose last reduce+2-matmul chain is short
- tail: ACT sqrt + DVE recip + one full-width ACT scale, transposes on
  TensorE, glc/gl copies on DVE, final-AG trigger ~7us after the last
  byte; all sim matmuls and the mid-chunk gather loads run inside the
  final AG wait; one row-split output store at the end."""

import sys

if "/opt/trn_rl_repo" not in sys.path:
    sys.path.insert(0, "/opt/trn_rl_repo")

import numpy as np

B_FULL = 512
C_IN = 2048
T_POOL = 196
O_OUT = 512
N_CORES = 8

CKS = [24, 32, 8]        # batches per chunk; one AllGather per chunk
                         # (early first chunk -> AG_a triggers ~150us, so
                         # even a pathologically slow AG_a completes before
                         # AG_b's data is ready; the exposed final AG
                         # carries only 8KB -- within-run data shows AG
                         # duration scales ~0.65x per halving)
JSPLIT = 2               # trailing batches of the LAST chunk streamed j-major
JGROUPS = [4, 4, 4, 2, 2]  # j-group unit sizes for the j-major tail


def build_kernel(b_full, c_in, t_pool, o_out, n_cores, ft_bufs=8):
    import concourse.mybir as mybir
    import concourse.tile as tile
    from concourse import bacc
    from concourse.masks import make_identity

    f32 = mybir.dt.float32
    bf16 = mybir.dt.bfloat16
    AL = mybir.AluOpType
    AF = mybir.ActivationFunctionType
    X = mybir.AxisListType.X

    bc = b_full // n_cores
    nj = 16
    cks = list(CKS)
    nch = len(cks)
    offs = [sum(cks[:c]) for c in range(nch)]
    ckmax = max(cks)
    oc = o_out // 128
    nr = n_cores
    assert c_in == 128 * nj and sum(cks) == bc and o_out % 128 == 0

    nc = bacc.Bacc("TRN2", target_bir_lowering=False, debug=False,
                   enable_asserts=False, num_devices=n_cores)
    feat = nc.dram_tensor("features", [bc, c_in, t_pool], f32,
                          kind="ExternalInput").ap()
    w_in = nc.dram_tensor("w", [o_out, c_in], f32, kind="ExternalInput").ap()
    bias_in = nc.dram_tensor("bias", [1, o_out], f32, kind="ExternalInput").ap()
    out_d = nc.dram_tensor("out", [bc, b_full], f32, kind="ExternalOutput").ap()

    with tile.TileContext(nc) as tc:
        with (
            tc.tile_pool(name="const", bufs=1) as constp,
            tc.tile_pool(name="wload", bufs=1) as wlp,
            tc.tile_pool(name="wtp", bufs=1) as wtp,
            tc.tile_pool(name="featp", bufs=ft_bufs) as fp,
            tc.tile_pool(name="featl", bufs=len(JGROUPS)) as flp,
            tc.tile_pool(name="poolp", bufs=1) as lp,
            tc.tile_pool(name="normp", bufs=2) as np_,
            tc.tile_pool(name="postp", bufs=1) as pp,
            tc.tile_pool(name="psrot", bufs=2, space="PSUM") as psp,
            tc.tile_pool(name="psgps", bufs=2, space="PSUM") as psgp,
            tc.tile_pool(name="pssim", bufs=2, space="PSUM") as pssp,
            tc.tile_pool(name="dram", bufs=1, space="DRAM") as dp,
        ):
            # ---- constants ----
            identf = constp.tile([128, 128], f32, name="identf")
            make_identity(nc, identf)
            identb = constp.tile([ckmax, ckmax], bf16, name="identb")
            make_identity(nc, identb)
            ones = constp.tile([1, ckmax], bf16, name="ones")
            nc.vector.memset(ones, 1.0)
            bias_sb = constp.tile([1, o_out], f32, name="bias_sb")
            nc.sync.dma_start(bias_sb[:], bias_in[:])
            bias_t = constp.tile([1, o_out], bf16, name="bias_t")
            nc.scalar.mul(bias_t[:], bias_sb[:], float(t_pool))

            # ---- W^T in bf16 (issued AFTER chunk-a's feature DMAs so
            # chunk-a completes ~11us earlier and every AG trigger gains
            # that much hiding slack; W still lands ~50us before the
            # first projection needs it) ----
            wt = []

            def load_w():
                wl = []
                for l in range(oc):
                    wli = wlp.tile([128, c_in], f32, name=f"wl{l}")
                    eng = nc.sync if l % 2 == 0 else nc.scalar
                    eng.dma_start(wli[:], w_in[l * 128:(l + 1) * 128, :])
                    wl.append(wli)
                for j in range(nj):
                    pswt = psp.tile([128, o_out], f32, name="pswt", tag="rot")
                    for l in range(oc):
                        src = wl[l][:, :].rearrange(
                            "o (p j) -> o p j", j=nj)[:, :, j]
                        nc.tensor.transpose(pswt[:, l * 128:(l + 1) * 128],
                                            src, identf[:])
                    wtj = wtp.tile([128, o_out], bf16, name=f"wt{j}")
                    nc.scalar.copy(wtj[:], pswt[:])
                    wt.append(wtj)

            gl_full = pp.tile([128, oc, bc], bf16, name="gl_full")
            outsb = pp.tile([bc, b_full], f32, name="outsb")
            glcs = [pp.tile([128, oc * cks[c]], bf16, name=f"glc{c}")
                    for c in range(nch)]
            agouts = []
            grts = []

            def pool_chunk(c, wcb=None):
                off, ck = offs[c], cks[c]
                p4 = lp.tile([128, ck, nj], bf16, name=f"p4_{c}")
                split = JSPLIT if c == nch - 1 else 0
                # the last 4 batches of EVERY chunk (and the whole
                # batch-major phase of the last chunk) stream at half-batch
                # granularity: DVE trails the stream by one delivery
                # quantum, so halving the quantum at each chunk boundary
                # halves the reduce backlog that delays the chunk's
                # projection/AG-trigger chain (and the final drain)
                nd = 0
                for i in range(ck - split):
                    b = off + i
                    if i == 8 and wcb is not None:
                        # W issues here: after the pool's 8 ungated feature
                        # issues (so W's ring slot isn't starved behind the
                        # reduce-gated trickle) but before the chunk's tail
                        # (so chunk-a still completes ~10us early)
                        wcb()
                    hv = 2 if (c == nch - 1 or i >= ck - split - 4) else 1
                    for h in range(hv):
                        jn = nj // hv
                        ft = fp.tile([128, jn * t_pool], f32, name="ft")
                        src = feat[b:b + 1, :, :].rearrange(
                            "b (p j) t -> p (b j) t", j=nj)
                        dma_eng = nc.scalar if nd % 2 == 0 else nc.sync
                        nd += 1
                        dma_eng.dma_start(
                            ft[:].rearrange("p (j t) -> p j t", t=t_pool),
                            src[:, h * jn:(h + 1) * jn, :])
                        with nc.allow_low_precision("pooled bf16"):
                            nc.vector.reduce_sum(
                                p4[:, i, h * jn:(h + 1) * jn],
                                ft[:].rearrange("p (j t) -> p j t",
                                                t=t_pool),
                                axis=X)
                # last chunk: stream the trailing batches j-group-major.
                # One DMA + ONE multi-batch reduce per j-group unit: the
                # reduce has 4x less DVE instruction overhead than per-batch
                # slices, so DVE keeps pace with arrival and the post-stream
                # drain is just the final (small) unit. Final units cover
                # only 2 j's so the last reduce+matmul chain is short.
                if split:
                    b0 = off + ck - split
                    src4 = feat[b0:b0 + split, :, :].rearrange(
                        "b (p j) t -> p b j t", j=nj)
                    joff = 0
                    for n, gj in enumerate(JGROUPS):
                        ftj = flp.tile([128, split, gj, t_pool], f32,
                                       name="ftl")
                        eng = nc.scalar if n % 2 == 0 else nc.sync
                        eng.dma_start(ftj[:], src4[:, :, joff:joff + gj, :])
                        with nc.allow_low_precision("pooled bf16"):
                            nc.vector.reduce_sum(
                                p4[:, ck - split:ck, joff:joff + gj],
                                ftj[:], axis=X)
                        joff += gj
                return p4

            def project(c, p4):
                ck = cks[c]
                gps = psgp.tile([ck, o_out], f32, name="gps", tag="gps")
                # bias accumulates FIRST: it has no data dependency on the
                # stream, so the last j-group matmul (on the AG trigger
                # chain) is also the accumulation stop
                nc.tensor.matmul(gps[:], ones[:, :ck], bias_t[:],
                                 start=True, stop=False)
                for j in range(nj):
                    nc.tensor.matmul(gps[:], p4[:, :, j], wt[j][:],
                                     start=False, stop=(j == nj - 1))
                return gps

            def start_ag(c, dma_eng):
                ck = cks[c]
                agin = dp.tile([128, oc * ck], bf16, name=f"agin{c}")
                agout = dp.tile([nr * 128, oc * ck], bf16, name=f"agout{c}",
                                addr_space="Shared")
                dma_eng.dma_start(agin[:], glcs[c][:])
                nc.gpsimd.collective_compute(
                    "AllGather", AL.bypass,
                    replica_groups=[list(range(n_cores))],
                    ins=[agin.opt()], outs=[agout.opt()],
                )
                agouts.append(agout)

            # ================= mid chunks =================
            for c in range(nch - 1):
                ck, off = cks[c], offs[c]
                p4 = pool_chunk(c, wcb=load_w if c == 0 else None)
                gps = project(c, p4)
                scr = np_.tile([ck, o_out], f32, name="scr")
                n2 = np_.tile([ck, 1], f32, name="n2")
                nc.scalar.activation(scr[:], gps[:], AF.Square,
                                     accum_out=n2[:])
                gsb = np_.tile([ck, o_out], f32, name="gsb")
                nc.scalar.copy(gsb[:], gps[:])
                nrm = np_.tile([ck, 1], f32, name="nrm")
                nc.scalar.sqrt(nrm[:], n2[:])
                gn = np_.tile([ck, o_out], bf16, name="gn")
                nc.gpsimd.normalize_recip(gn[:], gsb[:], nrm[:])
                glc_v = glcs[c][:].rearrange("p (m i) -> p m i", i=ck)
                for m in range(oc):
                    psg = psp.tile([128, ck], bf16, name="psg", tag="rot")
                    nc.tensor.transpose(psg[:], gn[:, m * 128:(m + 1) * 128],
                                        identb[:ck, :ck])
                    nc.scalar.copy(gl_full[:, m, off:off + ck], psg[:])
                    nc.scalar.copy(glc_v[:, m, :], psg[:])
                # collective stays on the SWDGE ring so the HWDGE feature
                # stream is never queued behind it; the gather LOAD is
                # deferred (see below) so the in-order gpsimd queue never
                # blocks the next chunk's normalize/trigger on a collective
                # completion
                start_ag(c, nc.gpsimd)

            # ================= last chunk =================
            cl = nch - 1
            ck, off = cks[cl], offs[cl]
            p4 = pool_chunk(cl)
            gps = project(cl, p4)

            # mid-chunk gather loads, deferred: AG_a is long done, so these
            # run mid-drain on the idle gpsimd ring; a late AG_b only delays
            # the final trigger in runs where ncfw would stall it anyway
            for c in range(nch - 1):
                ckc = cks[c]
                grt = pp.tile([128, nr, oc * ckc], bf16, name=f"grt{c}")
                nc.gpsimd.dma_start(
                    grt[:],
                    agouts[c][:, :].rearrange("(r p) f -> p r f", r=nr))
                grts.append(grt)

            scr1 = np_.tile([ck, o_out], f32, name="scr")
            n21 = np_.tile([ck, 1], f32, name="n2")
            nc.scalar.activation(scr1[:], gps[:], AF.Square, accum_out=n21[:])
            nrm1 = np_.tile([ck, 1], f32, name="nrm")
            nc.scalar.sqrt(nrm1[:], n21[:])
            rinv1 = pp.tile([ck, 1], f32, name="rinv1")
            nc.vector.reciprocal(rinv1[:], nrm1[:])
            gn1 = np_.tile([ck, o_out], bf16, name="gn")
            glc_v1 = glcs[cl][:].rearrange("p (m i) -> p m i", i=ck)
            # one full-width scale (per-instruction overhead dominates at
            # this size), then transposes + DVE copies pipeline per block
            nc.scalar.mul(gn1[:], gps[:], rinv1[:])
            for m in range(oc):
                psg = psp.tile([128, ck], bf16, name="psg", tag="rot")
                nc.tensor.transpose(psg[:], gn1[:, m * 128:(m + 1) * 128],
                                    identb[:ck, :ck])
                nc.vector.tensor_copy(glc_v1[:, m, :], psg[:])
                nc.vector.tensor_copy(gl_full[:, m, off:off + ck], psg[:])
            # trigger the final AG ASAP; sync ring is idle once the stream
            # is done
            start_ag(cl, nc.sync)

            # ---- sims for the mid chunks run during the final AG wait ----
            osb_rv = outsb[:, :].rearrange("b (r i) -> b r i", i=bc)
            for c in range(nch - 1):
                ckc, offc = cks[c], offs[c]
                simps = pssp.tile([bc, nr * ckc], f32, name="simps",
                                  tag=f"sim{c % 2}")
                for m in range(oc):
                    nc.tensor.matmul(
                        simps[:], gl_full[:, m, :],
                        grts[c][:, :, m * ckc:(m + 1) * ckc],
                        start=(m == 0), stop=(m == oc - 1))
                nc.vector.tensor_copy(osb_rv[:, :, offc:offc + ckc],
                                      simps[:])

            # ---- final chunk: split gather load, per-half sims ----
            grtl = pp.tile([128, nr, oc * ck], bf16, name=f"grt{cl}")
            agvl = agouts[cl][:, :].rearrange("(r p) f -> p r f", r=nr)
            hr = nr // 2
            nc.sync.dma_start(grtl[:, :hr, :], agvl[:, :hr, :])
            nc.scalar.dma_start(grtl[:, hr:, :], agvl[:, hr:, :])
            ck, off = cks[cl], offs[cl]
            for h in range(2):
                sph = pssp.tile([bc, hr * ck], f32, name=f"simh{h}",
                                tag=f"sim{h}")
                for m in range(oc):
                    nc.tensor.matmul(
                        sph[:], gl_full[:, m, :],
                        grtl[:, h * hr:(h + 1) * hr, m * ck:(m + 1) * ck],
                        start=(m == 0), stop=(m == oc - 1))
                nc.vector.tensor_copy(
                    osb_rv[:, h * hr:(h + 1) * hr, off:off + ck], sph[:])

            # row-split the output store across both idle rings
            nc.sync.dma_start(out_d[:bc // 2, :], outsb[:bc // 2, :])
            nc.scalar.dma_start(out_d[bc // 2:, :], outsb[bc // 2:, :])

    nc.compile()
    return nc


_NC_CACHE = {}


def _get_nc():
    key = (B_FULL, C_IN, T_POOL, O_OUT, N_CORES)
    if key not in _NC_CACHE:
        _NC_CACHE[key] = build_kernel(*key)
    return _NC_CACHE[key]


def _run(features, W, bias, trace=False, tmpdir=None):
    from concourse.bass_utils import run_bass_kernel_spmd

    feats = np.ascontiguousarray(np.asarray(features, dtype=np.float32))
    w_np = np.ascontiguousarray(np.asarray(W, dtype=np.float32))
    bias_np = np.ascontiguousarray(
        np.asarray(bias, dtype=np.float32).reshape(1, O_OUT))
    bc = B_FULL // N_CORES

    nc = _get_nc()
    in_maps = [
        {"features": feats[r * bc:(r + 1) * bc], "w": w_np, "bias": bias_np}
        for r in range(N_CORES)
    ]
    kw = {"tmpdir": tmpdir} if tmpdir else {}
    res = run_bass_kernel_spmd(nc, in_maps, core_ids=list(range(N_CORES)),
                               trace=trace, **kw)
    out = np.concatenate([res.results[r]["out"] for r in range(N_CORES)], axis=0)
    return out, res.exec_time_ns


def kernel(features, W, bias):
    out, _ = _run(features, W, bias)
    return out


# revision 29
# speedup vs baseline: 1.0697x; 1.0697x over previous
"""Chunked bf16 kernel with per-chunk ncfw AllGathers (16/32/16 batches).

Design (from r1-r8 NTFF traces):
- the feature stream runs near the per-NC HBM cap when nothing gates it:
  per-batch DMAs alternate the two HWDGE rings; nothing
  collective-adjacent ever rides those rings mid-stream (ring FIFO would
  stall the stream on a data dependency)
- ncfw mesh AllGather cost grows steeply with payload and has huge
  run-to-run variance (16KB: 5-26us, 32KB: 10-127us), so: a SMALL first
  chunk triggers AG_a by ~120us (even a pathological AG_a finishes
  before AG_b's data exists -- collective_compute blocks the issuing
  gpsimd engine until completion, so a late AG cascades), and the
  exposed final AG carries only 16KB
- DVE reduce throughput is layout-sensitive (ft pool needs 8 bufs for
  1.04 cyc/elem) and DVE trails the stream by ~one delivery quantum, so
  the last chunk streams half-batches and finishes with multi-batch
  j-group units ([4,4,4,2,2]) whose last reduce+2-matmul chain is short
- tail: ACT sqrt + DVE recip + one full-width ACT scale, transposes on
  TensorE, glc/gl copies on DVE, final-AG trigger ~7us after the last
  byte; all sim matmuls and the mid-chunk gather loads run inside the
  final AG wait; one row-split output store at the end."""

import sys

if "/opt/trn_rl_repo" not in sys.path:
    sys.path.insert(0, "/opt/trn_rl_repo")

import numpy as np

B_FULL = 512
C_IN = 2048
T_POOL = 196
O_OUT = 512
N_CORES = 8

CKS = [16, 32, 8, 8]     # batches per chunk; one AllGather per chunk
                         # (small first chunk -> AG_a triggers ~130us and
                         # AG_b ~290us, both hidden even when slow; the
                         # 8KB AG_c hides at ~310us; the exposed final AG
                         # carries only 8KB -- within-run data shows AG
                         # duration scales ~0.65x per payload halving)
JSPLIT = 2               # trailing batches of the LAST chunk streamed j-major
JGROUPS = [4, 4, 4, 2, 2]  # j-group unit sizes for the j-major tail


def build_kernel(b_full, c_in, t_pool, o_out, n_cores, ft_bufs=8):
    import concourse.mybir as mybir
    import concourse.tile as tile
    from concourse import bacc
    from concourse.masks import make_identity

    f32 = mybir.dt.float32
    bf16 = mybir.dt.bfloat16
    AL = mybir.AluOpType
    AF = mybir.ActivationFunctionType
    X = mybir.AxisListType.X

    bc = b_full // n_cores
    nj = 16
    cks = list(CKS)
    nch = len(cks)
    offs = [sum(cks[:c]) for c in range(nch)]
    ckmax = max(cks)
    oc = o_out // 128
    nr = n_cores
    assert c_in == 128 * nj and sum(cks) == bc and o_out % 128 == 0

    nc = bacc.Bacc("TRN2", target_bir_lowering=False, debug=False,
                   enable_asserts=False, num_devices=n_cores)
    feat = nc.dram_tensor("features", [bc, c_in, t_pool], f32,
                          kind="ExternalInput").ap()
    w_in = nc.dram_tensor("w", [o_out, c_in], f32, kind="ExternalInput").ap()
    bias_in = nc.dram_tensor("bias", [1, o_out], f32, kind="ExternalInput").ap()
    out_d = nc.dram_tensor("out", [bc, b_full], f32, kind="ExternalOutput").ap()

    with tile.TileContext(nc) as tc:
        with (
            tc.tile_pool(name="const", bufs=1) as constp,
            tc.tile_pool(name="wload", bufs=1) as wlp,
            tc.tile_pool(name="wtp", bufs=1) as wtp,
            tc.tile_pool(name="featp", bufs=ft_bufs) as fp,
            tc.tile_pool(name="featl", bufs=len(JGROUPS)) as flp,
            tc.tile_pool(name="poolp", bufs=1) as lp,
            tc.tile_pool(name="normp", bufs=2) as np_,
            tc.tile_pool(name="postp", bufs=1) as pp,
            tc.tile_pool(name="psrot", bufs=2, space="PSUM") as psp,
            tc.tile_pool(name="psgps", bufs=2, space="PSUM") as psgp,
            tc.tile_pool(name="pssim", bufs=2, space="PSUM") as pssp,
            tc.tile_pool(name="dram", bufs=1, space="DRAM") as dp,
        ):
            # ---- constants ----
            identf = constp.tile([128, 128], f32, name="identf")
            make_identity(nc, identf)
            identb = constp.tile([ckmax, ckmax], bf16, name="identb")
            make_identity(nc, identb)
            ones = constp.tile([1, ckmax], bf16, name="ones")
            nc.vector.memset(ones, 1.0)
            bias_sb = constp.tile([1, o_out], f32, name="bias_sb")
            nc.sync.dma_start(bias_sb[:], bias_in[:])
            bias_t = constp.tile([1, o_out], bf16, name="bias_t")
            nc.scalar.mul(bias_t[:], bias_sb[:], float(t_pool))

            # ---- W^T in bf16 (issued AFTER chunk-a's feature DMAs so
            # chunk-a completes ~11us earlier and every AG trigger gains
            # that much hiding slack; W still lands ~50us before the
            # first projection needs it) ----
            wt = []

            def load_w():
                wl = []
                for l in range(oc):
                    wli = wlp.tile([128, c_in], f32, name=f"wl{l}")
                    eng = nc.sync if l % 2 == 0 else nc.scalar
                    eng.dma_start(wli[:], w_in[l * 128:(l + 1) * 128, :])
                    wl.append(wli)
                for j in range(nj):
                    pswt = psp.tile([128, o_out], f32, name="pswt", tag="rot")
                    for l in range(oc):
                        src = wl[l][:, :].rearrange(
                            "o (p j) -> o p j", j=nj)[:, :, j]
                        nc.tensor.transpose(pswt[:, l * 128:(l + 1) * 128],
                                            src, identf[:])
                    wtj = wtp.tile([128, o_out], bf16, name=f"wt{j}")
                    nc.scalar.copy(wtj[:], pswt[:])
                    wt.append(wtj)

            gl_full = pp.tile([128, oc, bc], bf16, name="gl_full")
            outsb = pp.tile([bc, b_full], f32, name="outsb")
            glcs = [pp.tile([128, oc * cks[c]], bf16, name=f"glc{c}")
                    for c in range(nch)]
            agouts = []
            grts = []

            def pool_chunk(c, wcb=None):
                off, ck = offs[c], cks[c]
                p4 = lp.tile([128, ck, nj], bf16, name=f"p4_{c}")
                split = JSPLIT if c == nch - 1 else 0
                # the last 4 batches of EVERY chunk (and the whole
                # batch-major phase of the last chunk) stream at half-batch
                # granularity: DVE trails the stream by one delivery
                # quantum, so halving the quantum at each chunk boundary
                # halves the reduce backlog that delays the chunk's
                # projection/AG-trigger chain (and the final drain)
                nd = 0
                for i in range(ck - split):
                    b = off + i
                    if i == 8 and wcb is not None:
                        # W issues here: after the pool's 8 ungated feature
                        # issues (so W's ring slot isn't starved behind the
                        # reduce-gated trickle) but before the chunk's tail
                        # (so chunk-a still completes ~10us early)
                        wcb()
                    hv = 2 if (c == nch - 1 or i >= ck - split - 4) else 1
                    for h in range(hv):
                        jn = nj // hv
                        ft = fp.tile([128, jn * t_pool], f32, name="ft")
                        src = feat[b:b + 1, :, :].rearrange(
                            "b (p j) t -> p (b j) t", j=nj)
                        dma_eng = nc.scalar if nd % 2 == 0 else nc.sync
                        nd += 1
                        dma_eng.dma_start(
                            ft[:].rearrange("p (j t) -> p j t", t=t_pool),
                            src[:, h * jn:(h + 1) * jn, :])
                        with nc.allow_low_precision("pooled bf16"):
                            nc.vector.reduce_sum(
                                p4[:, i, h * jn:(h + 1) * jn],
                                ft[:].rearrange("p (j t) -> p j t",
                                                t=t_pool),
                                axis=X)
                # last chunk: stream the trailing batches j-group-major.
                # One DMA + ONE multi-batch reduce per j-group unit: the
                # reduce has 4x less DVE instruction overhead than per-batch
                # slices, so DVE keeps pace with arrival and the post-stream
                # drain is just the final (small) unit. Final units cover
                # only 2 j's so the last reduce+matmul chain is short.
                if split:
                    b0 = off + ck - split
                    src4 = feat[b0:b0 + split, :, :].rearrange(
                        "b (p j) t -> p b j t", j=nj)
                    joff = 0
                    for n, gj in enumerate(JGROUPS):
                        ftj = flp.tile([128, split, gj, t_pool], f32,
                                       name="ftl")
                        eng = nc.scalar if n % 2 == 0 else nc.sync
                        eng.dma_start(ftj[:], src4[:, :, joff:joff + gj, :])
                        with nc.allow_low_precision("pooled bf16"):
                            nc.vector.reduce_sum(
                                p4[:, ck - split:ck, joff:joff + gj],
                                ftj[:], axis=X)
                        joff += gj
                return p4

            def project(c, p4):
                ck = cks[c]
                gps = psgp.tile([ck, o_out], f32, name="gps", tag="gps")
                # bias accumulates FIRST: it has no data dependency on the
                # stream, so the last j-group matmul (on the AG trigger
                # chain) is also the accumulation stop
                nc.tensor.matmul(gps[:], ones[:, :ck], bias_t[:],
                                 start=True, stop=False)
                for j in range(nj):
                    nc.tensor.matmul(gps[:], p4[:, :, j], wt[j][:],
                                     start=False, stop=(j == nj - 1))
                return gps

            def start_ag(c, dma_eng):
                ck = cks[c]
                agin = dp.tile([128, oc * ck], bf16, name=f"agin{c}")
                agout = dp.tile([nr * 128, oc * ck], bf16, name=f"agout{c}",
                                addr_space="Shared")
                dma_eng.dma_start(agin[:], glcs[c][:])
                nc.gpsimd.collective_compute(
                    "AllGather", AL.bypass,
                    replica_groups=[list(range(n_cores))],
                    ins=[agin.opt()], outs=[agout.opt()],
                )
                agouts.append(agout)

            # ================= mid chunks =================
            for c in range(nch - 1):
                ck, off = cks[c], offs[c]
                p4 = pool_chunk(c, wcb=load_w if c == 0 else None)
                gps = project(c, p4)
                scr = np_.tile([ck, o_out], f32, name="scr")
                n2 = np_.tile([ck, 1], f32, name="n2")
                nc.scalar.activation(scr[:], gps[:], AF.Square,
                                     accum_out=n2[:])
                gsb = np_.tile([ck, o_out], f32, name="gsb")
                nc.scalar.copy(gsb[:], gps[:])
                nrm = np_.tile([ck, 1], f32, name="nrm")
                nc.scalar.sqrt(nrm[:], n2[:])
                gn = np_.tile([ck, o_out], bf16, name="gn")
                nc.gpsimd.normalize_recip(gn[:], gsb[:], nrm[:])
                glc_v = glcs[c][:].rearrange("p (m i) -> p m i", i=ck)
                for m in range(oc):
                    psg = psp.tile([128, ck], bf16, name="psg", tag="rot")
                    nc.tensor.transpose(psg[:], gn[:, m * 128:(m + 1) * 128],
                                        identb[:ck, :ck])
                    nc.scalar.copy(gl_full[:, m, off:off + ck], psg[:])
                    nc.scalar.copy(glc_v[:, m, :], psg[:])
                # collective stays on the SWDGE ring so the HWDGE feature
                # stream is never queued behind it; the gather LOAD is
                # deferred (see below) so the in-order gpsimd queue never
                # blocks the next chunk's normalize/trigger on a collective
                # completion
                start_ag(c, nc.gpsimd)

            # ================= last chunk =================
            cl = nch - 1
            ck, off = cks[cl], offs[cl]
            p4 = pool_chunk(cl)
            gps = project(cl, p4)

            # mid-chunk gather loads, deferred: AG_a is long done, so these
            # run mid-drain on the idle gpsimd ring; a late AG_b only delays
            # the final trigger in runs where ncfw would stall it anyway
            for c in range(nch - 1):
                ckc = cks[c]
                grt = pp.tile([128, nr, oc * ckc], bf16, name=f"grt{c}")
                nc.gpsimd.dma_start(
                    grt[:],
                    agouts[c][:, :].rearrange("(r p) f -> p r f", r=nr))
                grts.append(grt)

            scr1 = np_.tile([ck, o_out], f32, name="scr")
            n21 = np_.tile([ck, 1], f32, name="n2")
            nc.scalar.activation(scr1[:], gps[:], AF.Square, accum_out=n21[:])
            nrm1 = np_.tile([ck, 1], f32, name="nrm")
            nc.scalar.sqrt(nrm1[:], n21[:])
            rinv1 = pp.tile([ck, 1], f32, name="rinv1")
            nc.vector.reciprocal(rinv1[:], nrm1[:])
            gn1 = np_.tile([ck, o_out], bf16, name="gn")
            glc_v1 = glcs[cl][:].rearrange("p (m i) -> p m i", i=ck)
            # one full-width scale (per-instruction overhead dominates at
            # this size), then transposes + DVE copies pipeline per block
            nc.scalar.mul(gn1[:], gps[:], rinv1[:])
            for m in range(oc):
                psg = psp.tile([128, ck], bf16, name="psg", tag="rot")
                nc.tensor.transpose(psg[:], gn1[:, m * 128:(m + 1) * 128],
                                    identb[:ck, :ck])
                nc.vector.tensor_copy(glc_v1[:, m, :], psg[:])
                nc.vector.tensor_copy(gl_full[:, m, off:off + ck], psg[:])
            # trigger the final AG ASAP; sync ring is idle once the stream
            # is done
            start_ag(cl, nc.sync)

            # ---- sims for the mid chunks run during the final AG wait ----
            osb_rv = outsb[:, :].rearrange("b (r i) -> b r i", i=bc)
            for c in range(nch - 1):
                ckc, offc = cks[c], offs[c]
                simps = pssp.tile([bc, nr * ckc], f32, name="simps",
                                  tag=f"sim{c % 2}")
                for m in range(oc):
                    nc.tensor.matmul(
                        simps[:], gl_full[:, m, :],
                        grts[c][:, :, m * ckc:(m + 1) * ckc],
                        start=(m == 0), stop=(m == oc - 1))
                nc.vector.tensor_copy(osb_rv[:, :, offc:offc + ckc],
                                      simps[:])

            # ---- final chunk: split gather load, per-half sims ----
            grtl = pp.tile([128, nr, oc * ck], bf16, name=f"grt{cl}")
            agvl = agouts[cl][:, :].rearrange("(r p) f -> p r f", r=nr)
            hr = nr // 2
            nc.sync.dma_start(grtl[:, :hr, :], agvl[:, :hr, :])
            nc.scalar.dma_start(grtl[:, hr:, :], agvl[:, hr:, :])
            ck, off = cks[cl], offs[cl]
            for h in range(2):
                sph = pssp.tile([bc, hr * ck], f32, name=f"simh{h}",
                                tag=f"sim{h}")
                for m in range(oc):
                    nc.tensor.matmul(
                        sph[:], gl_full[:, m, :],
                        grtl[:, h * hr:(h + 1) * hr, m * ck:(m + 1) * ck],
                        start=(m == 0), stop=(m == oc - 1))
                nc.vector.tensor_copy(
                    osb_rv[:, h * hr:(h + 1) * hr, off:off + ck], sph[:])

            # row-split the output store across both idle rings
            nc.sync.dma_start(out_d[:bc // 2, :], outsb[:bc // 2, :])
            nc.scalar.dma_start(out_d[bc // 2:, :], outsb[bc // 2:, :])

    nc.compile()
    return nc


_NC_CACHE = {}


def _get_nc():
    key = (B_FULL, C_IN, T_POOL, O_OUT, N_CORES)
    if key not in _NC_CACHE:
        _NC_CACHE[key] = build_kernel(*key)
    return _NC_CACHE[key]


def _run(features, W, bias, trace=False, tmpdir=None):
    from concourse.bass_utils import run_bass_kernel_spmd

    feats = np.ascontiguousarray(np.asarray(features, dtype=np.float32))
    w_np = np.ascontiguousarray(np.asarray(W, dtype=np.float32))
    bias_np = np.ascontiguousarray(
        np.asarray(bias, dtype=np.float32).reshape(1, O_OUT))
    bc = B_FULL // N_CORES

    nc = _get_nc()
    in_maps = [
        {"features": feats[r * bc:(r + 1) * bc], "w": w_np, "bias": bias_np}
        for r in range(N_CORES)
    ]
    kw = {"tmpdir": tmpdir} if tmpdir else {}
    res = run_bass_kernel_spmd(nc, in_maps, core_ids=list(range(N_CORES)),
                               trace=trace, **kw)
    out = np.concatenate([res.results[r]["out"] for r in range(N_CORES)], axis=0)
    return out, res.exec_time_ns


def kernel(features, W, bias):
    out, _ = _run(features, W, bias)
    return out


# revision 32
# speedup vs baseline: 1.1112x; 1.0389x over previous
"""Chunked bf16 kernel with per-chunk ncfw AllGathers (16/32/16 batches).

Design (from r1-r8 NTFF traces):
- the feature stream runs near the per-NC HBM cap when nothing gates it:
  per-batch DMAs alternate the two HWDGE rings; nothing
  collective-adjacent ever rides those rings mid-stream (ring FIFO would
  stall the stream on a data dependency)
- ncfw mesh AllGather cost grows steeply with payload and has huge
  run-to-run variance (16KB: 5-26us, 32KB: 10-127us), so: a SMALL first
  chunk triggers AG_a by ~120us (even a pathological AG_a finishes
  before AG_b's data exists -- collective_compute blocks the issuing
  gpsimd engine until completion, so a late AG cascades), and the
  exposed final AG carries only 16KB
- DVE reduce throughput is layout-sensitive (ft pool needs 8 bufs for
  1.04 cyc/elem) and DVE trails the stream by ~one delivery quantum, so
  the last chunk streams half-batches and finishes with multi-batch
  j-group units ([4,4,4,2,2]) whose last reduce+2-matmul chain is short
- tail: ACT sqrt + DVE recip + one full-width ACT scale, transposes on
  TensorE, glc/gl copies on DVE, final-AG trigger ~7us after the last
  byte; all sim matmuls and the mid-chunk gather loads run inside the
  final AG wait; one row-split output store at the end."""

import sys

if "/opt/trn_rl_repo" not in sys.path:
    sys.path.insert(0, "/opt/trn_rl_repo")

import numpy as np

B_FULL = 512
C_IN = 2048
T_POOL = 196
O_OUT = 512
N_CORES = 8

CKS = [16, 32, 8, 8]     # batches per chunk; one AllGather per chunk
                         # (small first chunk -> AG_a triggers ~130us and
                         # AG_b ~290us, both hidden even when slow; the
                         # 8KB AG_c hides at ~310us; the exposed final AG
                         # carries only 8KB -- within-run data shows AG
                         # duration scales ~0.65x per payload halving)
JSPLIT = 2               # trailing batches of the LAST chunk streamed j-major
JGROUPS = [4, 4, 4, 2, 2]  # j-group unit sizes for the j-major tail


def build_kernel(b_full, c_in, t_pool, o_out, n_cores, ft_bufs=8):
    import concourse.mybir as mybir
    import concourse.tile as tile
    from concourse import bacc
    from concourse.masks import make_identity

    f32 = mybir.dt.float32
    bf16 = mybir.dt.bfloat16
    AL = mybir.AluOpType
    AF = mybir.ActivationFunctionType
    X = mybir.AxisListType.X

    bc = b_full // n_cores
    nj = 16
    cks = list(CKS)
    nch = len(cks)
    offs = [sum(cks[:c]) for c in range(nch)]
    ckmax = max(cks)
    oc = o_out // 128
    nr = n_cores
    assert c_in == 128 * nj and sum(cks) == bc and o_out % 128 == 0

    nc = bacc.Bacc("TRN2", target_bir_lowering=False, debug=False,
                   enable_asserts=False, num_devices=n_cores)
    feat = nc.dram_tensor("features", [bc, c_in, t_pool], f32,
                          kind="ExternalInput").ap()
    w_in = nc.dram_tensor("w", [o_out, c_in], f32, kind="ExternalInput").ap()
    bias_in = nc.dram_tensor("bias", [1, o_out], f32, kind="ExternalInput").ap()
    out_d = nc.dram_tensor("out", [bc, b_full], f32, kind="ExternalOutput").ap()

    with tile.TileContext(nc) as tc:
        with (
            tc.tile_pool(name="const", bufs=1) as constp,
            tc.tile_pool(name="wload", bufs=1) as wlp,
            tc.tile_pool(name="wtp", bufs=1) as wtp,
            tc.tile_pool(name="featp", bufs=ft_bufs) as fp,
            tc.tile_pool(name="featl", bufs=len(JGROUPS)) as flp,
            tc.tile_pool(name="poolp", bufs=1) as lp,
            tc.tile_pool(name="normp", bufs=2) as np_,
            tc.tile_pool(name="postp", bufs=1) as pp,
            tc.tile_pool(name="psrot", bufs=2, space="PSUM") as psp,
            tc.tile_pool(name="psgps", bufs=2, space="PSUM") as psgp,
            tc.tile_pool(name="pssim", bufs=2, space="PSUM") as pssp,
            tc.tile_pool(name="dram", bufs=1, space="DRAM") as dp,
        ):
            # ---- constants ----
            identf = constp.tile([128, 128], f32, name="identf")
            make_identity(nc, identf)
            identb = constp.tile([ckmax, ckmax], bf16, name="identb")
            make_identity(nc, identb)
            ones = constp.tile([1, ckmax], bf16, name="ones")
            nc.vector.memset(ones, 1.0)
            bias_sb = constp.tile([1, o_out], f32, name="bias_sb")
            nc.sync.dma_start(bias_sb[:], bias_in[:])
            bias_t = constp.tile([1, o_out], bf16, name="bias_t")
            nc.scalar.mul(bias_t[:], bias_sb[:], float(t_pool))

            # ---- W^T in bf16 (issued AFTER chunk-a's feature DMAs so
            # chunk-a completes ~11us earlier and every AG trigger gains
            # that much hiding slack; W still lands ~50us before the
            # first projection needs it) ----
            wt = []

            def load_w():
                wl = []
                for l in range(oc):
                    wli = wlp.tile([128, c_in], f32, name=f"wl{l}")
                    eng = nc.sync if l % 2 == 0 else nc.scalar
                    eng.dma_start(wli[:], w_in[l * 128:(l + 1) * 128, :])
                    wl.append(wli)
                for j in range(nj):
                    pswt = psp.tile([128, o_out], f32, name="pswt", tag="rot")
                    for l in range(oc):
                        src = wl[l][:, :].rearrange(
                            "o (p j) -> o p j", j=nj)[:, :, j]
                        nc.tensor.transpose(pswt[:, l * 128:(l + 1) * 128],
                                            src, identf[:])
                    wtj = wtp.tile([128, o_out], bf16, name=f"wt{j}")
                    nc.scalar.copy(wtj[:], pswt[:])
                    wt.append(wtj)

            gl_full = pp.tile([128, oc, bc], bf16, name="gl_full")
            outsb = pp.tile([bc, b_full], f32, name="outsb")
            glcs = [pp.tile([128, oc * cks[c]], bf16, name=f"glc{c}")
                    for c in range(nch)]
            agouts = []
            grts = []

            def pool_chunk(c, wcb=None):
                off, ck = offs[c], cks[c]
                p4 = lp.tile([128, ck, nj], bf16, name=f"p4_{c}")
                split = JSPLIT if c == nch - 1 else 0
                # the last 4 batches of EVERY chunk (and the whole
                # batch-major phase of the last chunk) stream at half-batch
                # granularity: DVE trails the stream by one delivery
                # quantum, so halving the quantum at each chunk boundary
                # halves the reduce backlog that delays the chunk's
                # projection/AG-trigger chain (and the final drain)
                nd = 0
                for i in range(ck - split):
                    b = off + i
                    if i == 8 and wcb is not None:
                        # W issues here: after the pool's 8 ungated feature
                        # issues (so W's ring slot isn't starved behind the
                        # reduce-gated trickle) but before the chunk's tail
                        # (so chunk-a still completes ~10us early)
                        wcb()
                    hv = 2 if (c == nch - 1 or i >= ck - split - 4) else 1
                    for h in range(hv):
                        jn = nj // hv
                        ft = fp.tile([128, jn * t_pool], f32, name="ft")
                        src = feat[b:b + 1, :, :].rearrange(
                            "b (p j) t -> p (b j) t", j=nj)
                        dma_eng = nc.scalar if nd % 2 == 0 else nc.sync
                        nd += 1
                        dma_eng.dma_start(
                            ft[:].rearrange("p (j t) -> p j t", t=t_pool),
                            src[:, h * jn:(h + 1) * jn, :])
                        with nc.allow_low_precision("pooled bf16"):
                            nc.vector.reduce_sum(
                                p4[:, i, h * jn:(h + 1) * jn],
                                ft[:].rearrange("p (j t) -> p j t",
                                                t=t_pool),
                                axis=X)
                # last chunk: stream the trailing batches j-group-major.
                # One DMA + ONE multi-batch reduce per j-group unit: the
                # reduce has 4x less DVE instruction overhead than per-batch
                # slices, so DVE keeps pace with arrival and the post-stream
                # drain is just the final (small) unit. Final units cover
                # only 2 j's so the last reduce+matmul chain is short.
                if split:
                    b0 = off + ck - split
                    src4 = feat[b0:b0 + split, :, :].rearrange(
                        "b (p j) t -> p b j t", j=nj)
                    joff = 0
                    for n, gj in enumerate(JGROUPS):
                        ftj = flp.tile([128, split, gj, t_pool], f32,
                                       name="ftl")
                        eng = nc.scalar if n % 2 == 0 else nc.sync
                        eng.dma_start(ftj[:], src4[:, :, joff:joff + gj, :])
                        with nc.allow_low_precision("pooled bf16"):
                            nc.vector.reduce_sum(
                                p4[:, ck - split:ck, joff:joff + gj],
                                ftj[:], axis=X)
                        joff += gj
                return p4

            def project(c, p4):
                ck = cks[c]
                gps = psgp.tile([ck, o_out], f32, name="gps", tag="gps")
                # bias accumulates FIRST: it has no data dependency on the
                # stream, so the last j-group matmul (on the AG trigger
                # chain) is also the accumulation stop
                nc.tensor.matmul(gps[:], ones[:, :ck], bias_t[:],
                                 start=True, stop=False)
                for j in range(nj):
                    nc.tensor.matmul(gps[:], p4[:, :, j], wt[j][:],
                                     start=False, stop=(j == nj - 1))
                return gps

            agins = []

            def stage_agin(c, dma_eng):
                ck = cks[c]
                agin = dp.tile([128, oc * ck], bf16, name=f"agin{c}")
                agout = dp.tile([nr * 128, oc * ck], bf16, name=f"agout{c}",
                                addr_space="Shared")
                dma_eng.dma_start(agin[:], glcs[c][:])
                agins.append(agin)
                agouts.append(agout)

            def trigger_ag(c):
                nc.gpsimd.collective_compute(
                    "AllGather", AL.bypass,
                    replica_groups=[list(range(n_cores))],
                    ins=[agins[c].opt()], outs=[agouts[c].opt()],
                )

            def start_ag(c, dma_eng):
                stage_agin(c, dma_eng)
                trigger_ag(c)

            # ================= mid chunks =================
            for c in range(nch - 1):
                ck, off = cks[c], offs[c]
                p4 = pool_chunk(c, wcb=load_w if c == 0 else None)
                gps = project(c, p4)
                scr = np_.tile([ck, o_out], f32, name="scr")
                n2 = np_.tile([ck, 1], f32, name="n2")
                nc.scalar.activation(scr[:], gps[:], AF.Square,
                                     accum_out=n2[:])
                gsb = np_.tile([ck, o_out], f32, name="gsb")
                nc.scalar.copy(gsb[:], gps[:])
                nrm = np_.tile([ck, 1], f32, name="nrm")
                nc.scalar.sqrt(nrm[:], n2[:])
                gn = np_.tile([ck, o_out], bf16, name="gn")
                nc.gpsimd.normalize_recip(gn[:], gsb[:], nrm[:])
                glc_v = glcs[c][:].rearrange("p (m i) -> p m i", i=ck)
                for m in range(oc):
                    psg = psp.tile([128, ck], bf16, name="psg", tag="rot")
                    nc.tensor.transpose(psg[:], gn[:, m * 128:(m + 1) * 128],
                                        identb[:ck, :ck])
                    nc.scalar.copy(gl_full[:, m, off:off + ck], psg[:])
                    nc.scalar.copy(glc_v[:, m, :], psg[:])
                # collective stays on the SWDGE ring so the HWDGE feature
                # stream is never queued behind it; the gather LOAD is
                # deferred (see below) so the in-order gpsimd queue never
                # blocks the next chunk's normalize/trigger on a collective
                # completion. The last TWO mid chunks stage their agin
                # DMAs first and trigger back-to-back afterwards: the
                # in-order gpsimd engine blocks inside each trigger until
                # that AG completes, so staging before triggering lets
                # AG_c's doorbell fire the instant AG_b drains and AG_c
                # overlaps the post-stream drain instead of serializing
                # after it.
                if c < nch - 3:
                    start_ag(c, nc.gpsimd)
                else:
                    stage_agin(c, nc.gpsimd)
                    if c == nch - 2:
                        for cd in range(max(0, nch - 3), nch - 1):
                            trigger_ag(cd)

            # ================= last chunk =================
            cl = nch - 1
            ck, off = cks[cl], offs[cl]
            p4 = pool_chunk(cl)
            gps = project(cl, p4)

            # mid-chunk gather loads, deferred: AG_a is long done, so these
            # run mid-drain on the idle gpsimd ring; a late AG_b only delays
            # the final trigger in runs where ncfw would stall it anyway
            for c in range(nch - 1):
                ckc = cks[c]
                grt = pp.tile([128, nr, oc * ckc], bf16, name=f"grt{c}")
                nc.gpsimd.dma_start(
                    grt[:],
                    agouts[c][:, :].rearrange("(r p) f -> p r f", r=nr))
                grts.append(grt)

            scr1 = np_.tile([ck, o_out], f32, name="scr")
            n21 = np_.tile([ck, 1], f32, name="n2")
            nc.scalar.activation(scr1[:], gps[:], AF.Square, accum_out=n21[:])
            nrm1 = np_.tile([ck, 1], f32, name="nrm")
            nc.scalar.sqrt(nrm1[:], n21[:])
            rinv1 = pp.tile([ck, 1], f32, name="rinv1")
            nc.vector.reciprocal(rinv1[:], nrm1[:])
            gn1 = np_.tile([ck, o_out], bf16, name="gn")
            glc_v1 = glcs[cl][:].rearrange("p (m i) -> p m i", i=ck)
            # one full-width scale (per-instruction overhead dominates at
            # this size), then transposes + DVE copies pipeline per block
            nc.scalar.mul(gn1[:], gps[:], rinv1[:])
            for m in range(oc):
                psg = psp.tile([128, ck], bf16, name="psg", tag="rot")
                nc.tensor.transpose(psg[:], gn1[:, m * 128:(m + 1) * 128],
                                    identb[:ck, :ck])
                nc.vector.tensor_copy(glc_v1[:, m, :], psg[:])
                nc.vector.tensor_copy(gl_full[:, m, off:off + ck], psg[:])
            # trigger the final AG ASAP; sync ring is idle once the stream
            # is done
            start_ag(cl, nc.sync)

            # ---- sims for the mid chunks run during the final AG wait ----
            osb_rv = outsb[:, :].rearrange("b (r i) -> b r i", i=bc)
            for c in range(nch - 1):
                ckc, offc = cks[c], offs[c]
                simps = pssp.tile([bc, nr * ckc], f32, name="simps",
                                  tag=f"sim{c % 2}")
                for m in range(oc):
                    nc.tensor.matmul(
                        simps[:], gl_full[:, m, :],
                        grts[c][:, :, m * ckc:(m + 1) * ckc],
                        start=(m == 0), stop=(m == oc - 1))
                nc.vector.tensor_copy(osb_rv[:, :, offc:offc + ckc],
                                      simps[:])

            # ---- final chunk: split gather load, per-half sims ----
            grtl = pp.tile([128, nr, oc * ck], bf16, name=f"grt{cl}")
            agvl = agouts[cl][:, :].rearrange("(r p) f -> p r f", r=nr)
            hr = nr // 2
            nc.sync.dma_start(grtl[:, :hr, :], agvl[:, :hr, :])
            nc.scalar.dma_start(grtl[:, hr:, :], agvl[:, hr:, :])
            ck, off = cks[cl], offs[cl]
            for h in range(2):
                sph = pssp.tile([bc, hr * ck], f32, name=f"simh{h}",
                                tag=f"sim{h}")
                for m in range(oc):
                    nc.tensor.matmul(
                        sph[:], gl_full[:, m, :],
                        grtl[:, h * hr:(h + 1) * hr, m * ck:(m + 1) * ck],
                        start=(m == 0), stop=(m == oc - 1))
                nc.vector.tensor_copy(
                    osb_rv[:, h * hr:(h + 1) * hr, off:off + ck], sph[:])

            # row-split the output store across both idle rings
            nc.sync.dma_start(out_d[:bc // 2, :], outsb[:bc // 2, :])
            nc.scalar.dma_start(out_d[bc // 2:, :], outsb[bc // 2:, :])

    nc.compile()
    return nc


_NC_CACHE = {}


def _get_nc():
    key = (B_FULL, C_IN, T_POOL, O_OUT, N_CORES)
    if key not in _NC_CACHE:
        _NC_CACHE[key] = build_kernel(*key)
    return _NC_CACHE[key]


def _run(features, W, bias, trace=False, tmpdir=None):
    from concourse.bass_utils import run_bass_kernel_spmd

    feats = np.ascontiguousarray(np.asarray(features, dtype=np.float32))
    w_np = np.ascontiguousarray(np.asarray(W, dtype=np.float32))
    bias_np = np.ascontiguousarray(
        np.asarray(bias, dtype=np.float32).reshape(1, O_OUT))
    bc = B_FULL // N_CORES

    nc = _get_nc()
    in_maps = [
        {"features": feats[r * bc:(r + 1) * bc], "w": w_np, "bias": bias_np}
        for r in range(N_CORES)
    ]
    kw = {"tmpdir": tmpdir} if tmpdir else {}
    res = run_bass_kernel_spmd(nc, in_maps, core_ids=list(range(N_CORES)),
                               trace=trace, **kw)
    out = np.concatenate([res.results[r]["out"] for r in range(N_CORES)], axis=0)
    return out, res.exec_time_ns


def kernel(features, W, bias):
    out, _ = _run(features, W, bias)
    return out
